# revision 3
# baseline (speedup 1.0000x reference)
"""DeepseekV2 MLA decode (matrix-absorbed) on 8 Trainium2 NeuronCores.

Sharding (all big operands bf16 on the wire; fp32 PSUM accumulation):
  - W_DQ row-sharded (contraction) -> partial cQ -> AllReduce (49KB) -> RMSNorm
    computed redundantly on every core (ln_w is folded into W_QR/W_UQ_UK host-side).
  - W_QR / W_UQ_UK head-sharded (16 of 128 heads per core).
  - AllGather of q (per-core [8,16,576] bf16 -> [8,8,16,576]).
  - Attention sharded over kv_len (1024 of 8192 positions per core, all 128 heads).
    k-rope is applied host-side with *relative* positions (q stays un-roped:
    R(a)q . R(b)k = q . R(b-a)k), both deinterleaved into concat-halves.
    ckv is shipped in BOTH layouts (natural [kv,l] and transposed [l,kv]) so no
    on-chip transposes of the caches are needed; scores are computed directly
    in the transposed [kv, head] layout so exp output feeds the attn matmul
    without a probs transpose; the softmax partial sums come from a ones-column
    matmul.  Partial (attn, lsum) -> ReduceScatter(add) grouped by head-block.
  - W_UV_O row-sharded (same 16 heads), streamed bf16; final AllReduce [8,5120].
"""
import sys

if "/opt/trn_rl_repo" not in sys.path:
    sys.path.insert(0, "/opt/trn_rl_repo")

import numpy as np
import ml_dtypes

BF = ml_dtypes.bfloat16

N_CORES = 8
B = 8           # batch
H = 5120        # hidden
NH = 128        # heads
QLR = 1536      # q lora rank
ROPE = 64
KVLR = 512
KV = 8192
THETA = 10000.0
SCALE = 192.0 ** -0.5

HL = NH // N_CORES      # 16 local heads
KVL = KV // N_CORES     # 1024 local kv positions
HD = H // N_CORES       # 640 local hidden (stage-1 contraction shard)
KT = KVL // 128         # 8 kv tiles of 128 per core

_CACHE = {}


def build_nc(sim=False):
    import concourse.bacc as bacc
    import concourse.mybir as mybir
    import concourse.tile as tile

    F32 = mybir.dt.float32
    BF16 = mybir.dt.bfloat16
    AF = mybir.ActivationFunctionType

    nc = bacc.Bacc("TRN2", target_bir_lowering=False, debug=False,
                   num_devices=(1 if sim else N_CORES))

    # ---- per-core inputs (host-preprocessed layouts, bf16) ----
    hs = nc.dram_tensor("hs", [B, HD], BF16, kind="ExternalInput")
    wdq = nc.dram_tensor("wdq", [128, 5 * QLR], BF16, kind="ExternalInput")
    wqr = nc.dram_tensor("wqr", [2, 128, 12 * 512], BF16, kind="ExternalInput")
    wuk = nc.dram_tensor("wuk", [16, 128, 12 * 512], BF16, kind="ExternalInput")
    ckv = nc.dram_tensor("ckv", [B, 128, KT * KVLR], BF16, kind="ExternalInput")
    ckvt = nc.dram_tensor("ckvt", [B, 128, 4 * KVL], BF16, kind="ExternalInput")
    kpet = nc.dram_tensor("kpet", [ROPE, B * KVL], BF16, kind="ExternalInput")
    ident = nc.dram_tensor("ident", [128, 128], BF16, kind="ExternalInput")
    wuvo = nc.dram_tensor("wuvo", [HL * KVLR, H], BF16, kind="ExternalInput")
    out = nc.dram_tensor("out", [B, H], F32, kind="ExternalOutput")

    RG = [list(range(N_CORES))]

    def coll(kind, op, in_t, out_t):
        if not sim:
            nc.gpsimd.collective_compute(kind, op, replica_groups=RG,
                                         ins=[in_t.opt()], outs=[out_t.opt()])
        elif kind == "AllGather":
            nc.sync.dma_start(out_t[0], in_t[:])
        elif kind == "ReduceScatter":
            nc.sync.dma_start(out_t[:], in_t[0])
        else:
            nc.sync.dma_start(out_t[:], in_t[:])

    with tile.TileContext(nc) as tc:
        with (
            tc.tile_pool(name="const", bufs=1) as cpool,
            tc.tile_pool(name="dram", bufs=1, space="DRAM") as dram,
            tc.tile_pool(name="dram_sh", bufs=1, space="DRAM") as dram_sh,
            tc.tile_pool(name="wuvo_sb", bufs=8) as wvp,
            tc.tile_pool(name="tpack", bufs=2, space="PSUM") as tpp,
        ):
            idt = cpool.tile([128, 128], BF16)
            nc.sync.dma_start(idt[:], ident[:])
            eps = cpool.tile([8, 1], F32)
            nc.vector.memset(eps[:], 1e-6)
            ones = cpool.tile([128, 1], BF16)
            nc.vector.memset(ones[:], 1.0)

            # collective bounce buffers (outputs in Shared address space)
            cq_ar_in = dram.tile([B, QLR], F32)
            cq_ar_out = dram_sh.tile([B, QLR], F32, addr_space="Shared")
            q_ag_in = dram.tile([B, HL, KVLR + ROPE], BF16)
            q_ag_out = dram_sh.tile([N_CORES, B, HL, KVLR + ROPE], BF16,
                                    addr_space="Shared")
            at_rs_in = dram.tile([N_CORES, B, HL, KVLR + 1], F32)
            at_rs_out = dram.tile([B, HL, KVLR + 1], F32)
            o_ar_in = dram.tile([B, H], F32)
            o_ar_out = dram_sh.tile([B, H], F32, addr_space="Shared")

            # =========== Stage 1: cQ = rmsnorm(hs @ W_DQ) ===========
            with (
                tc.tile_pool(name="s1", bufs=1) as s1,
                tc.tile_pool(name="s12ps", bufs=1, space="PSUM") as s1ps,
                tc.tile_pool(name="qnps", bufs=2, space="PSUM") as qnps,
                tc.tile_pool(name="wuk_sb", bufs=4) as wkp,
            ):
                hs_sb = s1.tile([B, HD], BF16)
                nc.sync.dma_start(hs_sb[:], hs[:])
                wdq_sb = s1.tile([128, 5, QLR], BF16)
                nc.sync.dma_start(wdq_sb[:], wdq[:].rearrange("p (k j) -> p k j", k=5))
                hsT = s1.tile([128, 5, 8], BF16)
                for k in range(5):
                    tp = tpp.tile([128, 8], BF16, tag="tp")
                    nc.tensor.transpose(tp[:], hs_sb[:, k * 128:(k + 1) * 128], idt[0:8, 0:8])
                    nc.vector.tensor_copy(hsT[:, k, :], tp[:])
                cq_ps = s1ps.tile([8, QLR], F32)
                for n in range(3):
                    for k in range(5):
                        nc.tensor.matmul(
                            cq_ps[:, n * 512:(n + 1) * 512],
                            hsT[:, k, :],
                            wdq_sb[:, k, n * 512:(n + 1) * 512],
                            start=(k == 0), stop=(k == 4),
                        )
                cqraw = s1.tile([8, QLR], F32)
                nc.scalar.copy(cqraw[:], cq_ps[:])
                nc.sync.dma_start(cq_ar_in[:], cqraw[:])
                coll("AllReduce", mybir.AluOpType.add, cq_ar_in, cq_ar_out)
                cqsum = s1.tile([8, QLR], F32)
                nc.sync.dma_start(cqsum[:], cq_ar_out[:])
                # rmsnorm (ln_w folded into the weights host-side)
                sq = s1.tile([8, QLR], F32)
                ssq = s1.tile([8, 1], F32)
                nc.scalar.activation(sq[:], cqsum[:], AF.Square, accum_out=ssq[:])
                sdev = s1.tile([8, 1], F32)
                nc.scalar.activation(sdev[:], ssq[:], AF.Sqrt, bias=eps[:], scale=1.0 / QLR)
                rinv = s1.tile([8, 1], F32)
                nc.vector.reciprocal(rinv[:], sdev[:])
                cqn = s1.tile([8, QLR], BF16)
                nc.vector.tensor_scalar_mul(cqn[:], cqsum[:], rinv[:])
                cqnT = s1.tile([128, 12, 8], BF16)
                for k in range(12):
                    tp = tpp.tile([128, 8], BF16, tag="tp")
                    nc.tensor.transpose(tp[:], cqn[:, k * 128:(k + 1) * 128], idt[0:8, 0:8])
                    nc.vector.tensor_copy(cqnT[:, k, :], tp[:])

                # =========== Stage 2: q projections for 16 local heads ===========
                qpe_sb = s1.tile([8, HL * ROPE], F32)
                for n in range(2):
                    wt = wkp.tile([128, 12, 512], BF16, tag="wuk")
                    nc.sync.dma_start(wt[:], wqr[n].rearrange("p (k j) -> p k j", k=12))
                    ps_q = qnps.tile([8, 512], F32)
                    for k in range(12):
                        nc.tensor.matmul(ps_q[:], cqnT[:, k, :], wt[:, k, :],
                                         start=(k == 0), stop=(k == 11))
                    nc.scalar.copy(qpe_sb[:, n * 512:(n + 1) * 512], ps_q[:])
                qn_sb = s1.tile([8, HL, KVLR], BF16)
                for n in range(16):
                    wt = wkp.tile([128, 12, 512], BF16, tag="wuk")
                    nc.sync.dma_start(wt[:], wuk[n].rearrange("p (k j) -> p k j", k=12))
                    ps_q = qnps.tile([8, 512], F32)
                    for k in range(12):
                        nc.tensor.matmul(ps_q[:], cqnT[:, k, :], wt[:, k, :],
                                         start=(k == 0), stop=(k == 11))
                    nc.scalar.copy(qn_sb[:, n, :], ps_q[:])
                # deinterleave q_pe (concat-halves permutation, matching host k layout)
                qpe2 = s1.tile([8, HL, ROPE], BF16)
                qv = qpe_sb[:].rearrange("b (h r) -> b h r", h=HL)
                nc.vector.tensor_copy(qpe2[:, :, 0:32], qv[:, :, 0:ROPE:2])
                nc.vector.tensor_copy(qpe2[:, :, 32:64], qv[:, :, 1:ROPE:2])
                # pack q into the allgather buffer
                nc.sync.dma_start(q_ag_in[:, :, 0:KVLR], qn_sb[:])
                nc.sync.dma_start(q_ag_in[:, :, KVLR:KVLR + ROPE], qpe2[:])
                coll("AllGather", mybir.AluOpType.bypass, q_ag_in, q_ag_out)

            # =========== Stage 3: attention over local kv shard, all 128 heads ===========
            with (
                tc.tile_pool(name="s3", bufs=2) as s3,
                tc.tile_pool(name="kp", bufs=1) as kp,
                tc.tile_pool(name="scps", bufs=2, space="PSUM") as scps,
                tc.tile_pool(name="atps", bufs=1, space="PSUM") as atps,
                tc.tile_pool(name="lsps", bufs=1, space="PSUM") as lsps,
            ):
                kpet_sb = kp.tile([ROPE, B * KVL], BF16)
                nc.sync.dma_start(kpet_sb[:], kpet[:])
                for b in range(B):
                    qn_all = s3.tile([128, KVLR], BF16, tag="qn_all")
                    nc.sync.dma_start(qn_all[:], q_ag_out[:, b, :, 0:KVLR])
                    qe_all = s3.tile([128, ROPE], BF16, tag="qe_all")
                    nc.sync.dma_start(qe_all[:], q_ag_out[:, b, :, KVLR:KVLR + ROPE])
                    # transpose q:  qnT [l(4x128), h=128],  qeT [r=64, h=128]
                    qnT = s3.tile([128, 4, 128], BF16, tag="qnT")
                    tp = tpp.tile([128, 512], BF16, tag="tp")
                    for lc in range(4):
                        nc.tensor.transpose(tp[:, lc * 128:(lc + 1) * 128],
                                            qn_all[:, lc * 128:(lc + 1) * 128], idt[:])
                    nc.vector.tensor_copy(qnT[:].rearrange("p a b -> p (a b)"), tp[:])
                    qeT = s3.tile([64, 128], BF16, tag="qeT")
                    tpq = tpp.tile([64, 128], BF16, tag="tp")
                    nc.tensor.transpose(tpq[:], qe_all[:], idt[:])
                    nc.vector.tensor_copy(qeT[:], tpq[:])
                    # caches (both layouts shipped from host)
                    ckv_sb = s3.tile([128, KT, KVLR], BF16, tag="ckv")
                    nc.sync.dma_start(ckv_sb[:], ckv[b].rearrange("p (t l) -> p t l", t=KT))
                    ckvt_sb = s3.tile([128, 4, KVL], BF16, tag="ckvt")
                    nc.sync.dma_start(ckvt_sb[:], ckvt[b].rearrange("p (c k) -> p c k", c=4))
                    # scoresT [kv(8x128), h=128] = ckv . qn^T + k_roped . qe^T
                    sc_ps = scps.tile([128, KT, 128], F32)
                    for cc in range(KT):
                        for lc in range(4):
                            nc.tensor.matmul(sc_ps[:, cc, :],
                                             ckvt_sb[:, lc, cc * 128:(cc + 1) * 128],
                                             qnT[:, lc, :],
                                             start=(lc == 0), stop=False)
                        nc.tensor.matmul(sc_ps[:, cc, :],
                                         kpet_sb[:, b * KVL + cc * 128:b * KVL + (cc + 1) * 128],
                                         qeT[:],
                                         start=False, stop=True)
                    # probsT = exp(scoresT * SCALE)  (no max subtraction; scores O(7))
                    probsT = s3.tile([128, KT, 128], BF16, tag="probsT")
                    nc.scalar.activation(probsT[:].rearrange("p a b -> p (a b)"),
                                         sc_ps[:].rearrange("p a b -> p (a b)"),
                                         AF.Exp, scale=SCALE)
                    # attn partial [h=128, KVLR] and lsum partial [h=128, 1]
                    at_ps = atps.tile([128, KVLR], F32)
                    for cc in range(KT):
                        nc.tensor.matmul(at_ps[:], probsT[:, cc, :], ckv_sb[:, cc, :],
                                         start=(cc == 0), stop=(cc == KT - 1))
                    ls_ps = lsps.tile([128, 1], F32)
                    for cc in range(KT):
                        nc.tensor.matmul(ls_ps[:], probsT[:, cc, :], ones[:],
                                         start=(cc == 0), stop=(cc == KT - 1))
                    attn_sb = s3.tile([128, KVLR + 1], F32, tag="attn")
                    nc.vector.tensor_copy(attn_sb[:, 0:KVLR], at_ps[:])
                    nc.vector.tensor_copy(attn_sb[:, KVLR:KVLR + 1], ls_ps[:])
                    # one DMA per batch into the reduce buffer (head-block grouped)
                    nc.sync.dma_start(at_rs_in[:, b, :, :], attn_sb[:])
                coll("ReduceScatter", mybir.AluOpType.add, at_rs_in, at_rs_out)

            # =========== Stage 4: out = (attn/lsum) @ W_UV_O, head shard ===========
            with (
                tc.tile_pool(name="s4", bufs=1) as s4,
                tc.tile_pool(name="oaps", bufs=1, space="PSUM") as oaps,
            ):
                o_sb = s4.tile([8, HL, KVLR + 1], F32)
                nc.sync.dma_start(o_sb[:], at_rs_out[:])
                linv = s4.tile([8, HL], F32)
                nc.vector.reciprocal(linv[:], o_sb[:, :, KVLR])
                osc = s4.tile([8, HL, KVLR], BF16)
                for h in range(HL):
                    nc.vector.tensor_scalar_mul(osc[:, h, :], o_sb[:, h, 0:KVLR],
                                                linv[:, h:h + 1])
                aT = s4.tile([128, HL * 4, 8], BF16)
                for h in range(HL):
                    tp = tpp.tile([128, 32], BF16, tag="tp")
                    for lc in range(4):
                        nc.tensor.transpose(tp[:, lc * 8:(lc + 1) * 8],
                                            osc[:, h, lc * 128:(lc + 1) * 128], idt[0:8, 0:8])
                    nc.vector.tensor_copy(
                        aT[:, h * 4:(h + 1) * 4, :].rearrange("p a b -> p (a b)"), tp[:])
                outp = s4.tile([8, H], F32)
                NHALF = H // 2
                for half in range(2):
                    o_ps = oaps.tile([8, NHALF], F32)
                    for r in range(64):
                        wt = wvp.tile([128, NHALF], BF16, tag="wuvo")
                        nc.sync.dma_start(
                            wt[:], wuvo[r * 128:(r + 1) * 128,
                                        half * NHALF:(half + 1) * NHALF])
                        for n5 in range(NHALF // 512):
                            nc.tensor.matmul(o_ps[:, n5 * 512:(n5 + 1) * 512],
                                             aT[:, r, :], wt[:, n5 * 512:(n5 + 1) * 512],
                                             start=(r == 0), stop=(r == 63))
                    nc.scalar.copy(outp[:, half * NHALF:(half + 1) * NHALF], o_ps[:])
                nc.sync.dma_start(o_ar_in[:], outp[:])
                coll("AllReduce", mybir.AluOpType.add, o_ar_in, o_ar_out)
                nc.sync.dma_start(out[:], o_ar_out[:])

    nc.compile()
    return nc


def make_in_maps(hidden_states, compressed_kv_normed_cache, k_pe_cache,
                 W_DQ, ln_w, W_QR, W_UQ_UK, W_UV_O):
    f32 = np.float32
    hidden_states = np.asarray(hidden_states, f32)
    ckv = np.asarray(compressed_kv_normed_cache, f32)
    kpe = np.asarray(k_pe_cache, f32)
    W_DQ = np.asarray(W_DQ, f32)
    ln_w = np.asarray(ln_w, f32)
    W_QR = np.asarray(W_QR, f32) * ln_w[:, None]
    W_UQ_UK = np.asarray(W_UQ_UK, f32) * ln_w[:, None]
    W_UV_O = np.asarray(W_UV_O, f32)

    # host rope of the k cache at *relative* positions (q stays un-roped),
    # deinterleaved into concat-halves to match the on-chip q layout
    inv = 1.0 / (THETA ** (np.arange(0, ROPE, 2, dtype=np.float64) / ROPE))
    rel = (np.arange(KV, dtype=np.float64) - (KV - 1))[:, None] * inv[None, :]
    cosr = np.cos(rel).astype(f32)[None, :, :]
    sinr = np.sin(rel).astype(f32)[None, :, :]
    k0, k1 = kpe[:, :, 0::2], kpe[:, :, 1::2]
    kr = np.concatenate([k0 * cosr - k1 * sinr, k0 * sinr + k1 * cosr], axis=2)
    # [B, KV, ROPE] -> per-core transposed [ROPE, B*KVL]
    ident = np.eye(128, dtype=BF)

    c = np.ascontiguousarray

    def blk(w, n_k, n_n):
        # [K, N] -> [n_n, 128, n_k*nchunk] with p-major contraction blocks:
        # w_r[j, p, k*nch + i] = w[k*128 + p, j*nch + i]
        K, N = w.shape
        nch = N // n_n
        return c(w.reshape(n_k, 128, n_n, nch).transpose(2, 1, 0, 3)
                 .reshape(n_n, 128, n_k * nch).astype(BF))

    in_maps = []
    for ci in range(N_CORES):
        kvs = slice(ci * KVL, (ci + 1) * KVL)
        ckv_c = ckv[:, kvs, :]                       # [B, KVL, KVLR]
        wdq_c = W_DQ[ci * HD:(ci + 1) * HD, :]       # [640, QLR]
        in_maps.append({
            "hs": c(hidden_states[:, ci * HD:(ci + 1) * HD].astype(BF)),
            # [640,QLR] -> [128, 5*QLR]: p-major k blocks
            "wdq": c(wdq_c.reshape(5, 128, QLR).transpose(1, 0, 2)
                     .reshape(128, 5 * QLR).astype(BF)),
            "wqr": blk(W_QR[:, ci * HL * ROPE:(ci + 1) * HL * ROPE], 12, 2),
            "wuk": blk(W_UQ_UK[:, ci * HL * KVLR:(ci + 1) * HL * KVLR], 12, 16),
            # natural: [B, 128, KT*KVLR]  (kv tile-major, p-minor rows)
            "ckv": c(ckv_c.reshape(B, KT, 128, KVLR).transpose(0, 2, 1, 3)
                     .reshape(B, 128, KT * KVLR).astype(BF)),
            # transposed: [B, 128, 4*KVL]  (l = c*128+p, kv contiguous)
            "ckvt": c(ckv_c.transpose(0, 2, 1).reshape(B, 4, 128, KVL)
                      .transpose(0, 2, 1, 3).reshape(B, 128, 4 * KVL).astype(BF)),
            "kpet": c(kr[:, kvs, :].transpose(2, 0, 1).reshape(ROPE, B * KVL).astype(BF)),
            "ident": ident,
            "wuvo": c(W_UV_O[ci * HL * KVLR:(ci + 1) * HL * KVLR, :].astype(BF)),
        })
    return in_maps


def kernel(**inputs) -> np.ndarray:
    from concourse import bass_utils

    if "nc" not in _CACHE:
        _CACHE["nc"] = build_nc()
    nc = _CACHE["nc"]
    in_maps = make_in_maps(**inputs)
    res = bass_utils.run_bass_kernel_spmd(nc, in_maps, core_ids=list(range(N_CORES)))
    return np.asarray(res.results[0]["out"], np.float32)
